# revision 9
# baseline (speedup 1.0000x reference)
"""KAN layer TRN2 kernel v2.1 (data-parallel, 8 cores).

Identities:
    relu(w*x+b) = w * extremum(x - c, 0),  c = -b/w  (max if w>0 else min)
    sum_d extremum(x, c) = sum_d extremum(x-c, 0) + D*c

Per core (BL=512 rows, 4 row-tiles):
  - xp = x@wp+bp on PE (fp16, bp added during the PSUM->SBUF cast via a
    replicated-bp tensor_tensor add on DVE).
  - xpT (transposed xp, [d,b] pieces) built by DMA XBAR transposes of the
    xp_sb tiles on the SP queue -- zero compute-engine time.
  - inner h's split 3 ways (tunable):
      FD: DVE tensor_scalar CCE accum on xp_sb      (extremum(x,c) sums)
      FA: ACT activation(Relu)+accum on xp_ps PSUM  (true relu sums)
      TR: DVE 2x fp16 elementwise extremum(xpT-c,0), then PE matmuls with
          rank-1 dup'd wi2 lhsT reduce straight into uT_dup [128,512] PSUM
          (row p holds q = p mod 64; both halves identical).
    Fused R columns: junction-cast fp16 -> XBAR transpose -> one PE matmul.
  - u2 = fp16 copy of uT_dup with bias_u added per-partition (ACT).
  - outer: 64 h2 two-per-op on DVE (per-partition thresholds via AP scalar);
    reduction over q by PE one-hot-block matmuls into S_T [64,512] plus a
    few pairs partition-reduced on GpSimd straight into S_T_sb.
  - final: summed = S_T(+ones row).T @ [wo1*wo2; Q*bo2] per tile, LayerNorm.
"""

import numpy as np

import concourse.bass as bass
import concourse.tile as tile
from concourse import mybir
from concourse.bass_utils import run_bass_kernel_spmd

B, DIN, DOUT, Q, H1, H2 = 4096, 768, 512, 64, 32, 64
EPS = 1e-5
NCORES = 8
BL = B // NCORES   # 512
NT = BL // 128     # 4
KC = DIN // 128    # 6

# inner h assignment (tunable)
N_FD = 8   # DVE fused CCE
N_FA = 10  # ACT fused relu-accum
H_FD = list(range(0, N_FD))
H_FA = list(range(N_FD, N_FD + N_FA))
H_TR = list(range(N_FD + N_FA, H1))
N_TR = len(H_TR)
N_F = N_FD + N_FA

N_GP_PAIRS = 0  # outer pairs reduced on GpSimd instead of PE (0 = all on PE)

F32 = mybir.dt.float32
F16 = mybir.dt.float16
AF = mybir.ActivationFunctionType
OP = mybir.AluOpType


def _outer_pairs(wo1, bo1):
    c2 = -bo1 / wo1
    pos = [h for h in range(H2) if wo1[h] > 0]
    neg = [h for h in range(H2) if wo1[h] <= 0]
    pairs = []
    for grp, use_max in ((pos, True), (neg, False)):
        for i in range(0, len(grp) - 1, 2):
            pairs.append((grp[i], grp[i + 1], use_max, True))
        if len(grp) % 2:
            pairs.append((grp[-1], grp[-1], use_max, False))
    return pairs, c2


# ---- packed fp16 const-buffer column layout (host must mirror) ----
def _c16_layout(npair):
    off = {}
    o = 0
    off["w2dup"] = o; o += 128 * N_TR
    off["wi2fd"] = o; o += 128
    off["pmask"] = o; o += 64 * npair
    off["wo2e"] = o; o += DOUT
    off["total"] = o
    return off


def _build_program(wi1, bi1, wo1, bo1, trivial_ln, pairs):
    nc = bass.Bass()
    npair = len(pairs)
    L = _c16_layout(npair)

    xTall = nc.declare_dram_parameter("xTall", [128, KC * BL], F16,
                                      isOutput=False)
    wpall = nc.declare_dram_parameter("wpall", [128, KC * DOUT], F16,
                                      isOutput=False)
    c16 = nc.declare_dram_parameter("c16", [128, L["total"]], F16,
                                    isOutput=False)
    c32 = nc.declare_dram_parameter("c32", [128, 517 + npair], F32,
                                    isOutput=False)
    gamma = nc.declare_dram_parameter("gamma", [DOUT], F32, isOutput=False)
    beta = nc.declare_dram_parameter("beta", [DOUT], F32, isOutput=False)
    y = nc.declare_dram_parameter("y", [BL, DOUT], F16, isOutput=True)

    c_in = -bi1 / wi1

    from contextlib import ExitStack

    with tile.TileContext(nc) as tc, ExitStack() as ctx:
        singles = ctx.enter_context(tc.tile_pool(name="singles", bufs=1))
        s_pool = ctx.enter_context(tc.tile_pool(name="s_pool", bufs=8))
        s2_pool = ctx.enter_context(tc.tile_pool(name="s2_pool", bufs=8))
        scr_d = ctx.enter_context(tc.tile_pool(name="scr_d", bufs=2))
        scr_a = ctx.enter_context(tc.tile_pool(name="scr_a", bufs=2))
        rcat_pool = ctx.enter_context(tc.tile_pool(name="rcat", bufs=3))
        small = ctx.enter_context(tc.tile_pool(name="small", bufs=4))
        ypool = ctx.enter_context(tc.tile_pool(name="ypool", bufs=3))
        ps_xp = ctx.enter_context(tc.tile_pool(name="ps_xp", bufs=4,
                                               space="PSUM"))
        ps_big = ctx.enter_context(tc.tile_pool(name="ps_big", bufs=2,
                                                space="PSUM"))
        ps_st = ctx.enter_context(tc.tile_pool(name="ps_st", bufs=1,
                                               space="PSUM"))
        ps_u = ctx.enter_context(tc.tile_pool(name="ps_u", bufs=1,
                                              space="PSUM"))

        # ---------------- inputs / consts ----------------
        xT_sb = singles.tile([128, KC * BL], F16, tag="xTall")
        nc.sync.dma_start(out=xT_sb, in_=xTall[:, :])
        wp_sb = singles.tile([128, KC * DOUT], F16, tag="wpall")
        nc.scalar.dma_start(out=wp_sb, in_=wpall[:, :])
        c16_sb = singles.tile([128, L["total"]], F16, tag="c16")
        nc.scalar.dma_start(out=c16_sb, in_=c16[:, :])
        c32_sb = singles.tile([128, 517 + npair], F32, tag="c32")
        nc.sync.dma_start(out=c32_sb, in_=c32[:, :])
        bp_rep = c32_sb[:, 0:DOUT]
        if not trivial_ln:
            gam_rep = singles.tile([128, DOUT], F32, tag="gam")
            nc.gpsimd.dma_start(
                out=gam_rep,
                in_=bass.AP(tensor=gamma[:].tensor, offset=gamma[:].offset,
                            ap=[[0, 128]] + list(gamma[:].ap)))
            bet_rep = singles.tile([128, DOUT], F32, tag="bet")
            nc.gpsimd.dma_start(
                out=bet_rep,
                in_=bass.AP(tensor=beta[:].tensor, offset=beta[:].offset,
                            ap=[[0, 128]] + list(beta[:].ap)))

        w2d_sb = c16_sb[:, L["w2dup"]:L["w2dup"] + 128 * N_TR]
        wi2fd_sb = c16_sb[0:N_F, L["wi2fd"]:L["wi2fd"] + 128]
        pmask_sb = c16_sb[:, L["pmask"]:L["pmask"] + 64 * npair]
        wo2e_sb = c16_sb[0:H2 + 1, L["wo2e"]:L["wo2e"] + DOUT]
        biasud_col = c32_sb[:, 512:513]   # bias_u dup'd [128,1]
        cpair_cols = c32_sb[:, 513:513 + npair]
        bp4_cols = c32_sb[:, 513 + npair:517 + npair]

        eps_sb = singles.tile([128, 1], F32, tag="eps")
        nc.vector.memset(eps_sb, EPS)
        xp_sbs = singles.tile([128, NT * DOUT], F16, tag="xp_sb")
        xpT_sb = singles.tile([128, NT * BL], F16, tag="xpT")
        R_fT = singles.tile([128, BL], F16, tag="RfT")
        u2_sb = singles.tile([128, BL], F16, tag="u2")
        S_T_sb = singles.tile([H2 + 1, BL], F32, tag="ST")
        nc.vector.memset(S_T_sb[H2:H2 + 1, :], 1.0)
        S_T16 = singles.tile([H2 + 1, BL], F16, tag="ST16")
        nc.vector.memset(S_T16[H2:H2 + 1, :], 1.0)

        # per-FA-h bias columns (bi1 replicated) aren't needed: ACT bias can
        # be a [128,1] f32 AP -- pack those in c32? simpler: memset consts.
        bi1_cols = singles.tile([128, N_FA], F32, tag="bi1cols")
        for i, h in enumerate(H_FA):
            nc.vector.memset(bi1_cols[:, i:i + 1], float(bi1[h]))

        # ---------------- compute ----------------
        xp_ps = [ps_xp.tile([128, DOUT], F32, tag="xp_ps", name=f"xp_ps{j}")
                 for j in range(NT)]

        def emit_xp_mm(j):
            for c in range(KC):
                nc.tensor.matmul(
                    xp_ps[j], xT_sb[:, c * BL + j * 128:c * BL + (j + 1) * 128],
                    wp_sb[:, c * DOUT:(c + 1) * DOUT],
                    start=(c == 0), stop=(c == KC - 1))

        def xp_cast(j):
            # PSUM f32 + bp -> SBUF f16 (DVE)
            nc.vector.tensor_tensor(
                out=xp_sbs[:, j * DOUT:(j + 1) * DOUT], in0=xp_ps[j],
                in1=bp_rep, op=OP.add)

        def xbar_xp(j):
            # xpT blocks for piece 0 (pieces 1-3 come from PE matmuls)
            for p in range(1):
                nc.sync.dma_start(
                    out=xpT_sb[:, p * BL + j * 128:p * BL + (j + 1) * 128],
                    in_=xp_sbs[:, j * DOUT + p * 128:j * DOUT + (p + 1) * 128],
                    transpose=True)

        uT_ps = ps_u.tile([128, BL], F32, tag="uT")
        R_cats = []
        Rc16s = []

        def emit_fused(j):
            R_cat = rcat_pool.tile([128, N_F], F32, tag="R_cat")
            R_cats.append(R_cat)
            xp_sb_j = xp_sbs[:, j * DOUT:(j + 1) * DOUT]
            for i, h in enumerate(H_FD):
                s = scr_d.tile([128, DOUT], F16, tag="sd")
                op0 = OP.max if wi1[h] > 0 else OP.min
                nc.vector.tensor_scalar(
                    s, xp_sb_j, float(c_in[h]), 0.0, op0, OP.add,
                    accum_out=R_cat[:, i:i + 1])
            for i, h in enumerate(H_FA):
                s = scr_a.tile([128, DOUT], F16, tag="sa")
                nc.scalar.activation(
                    s, xp_ps[j], AF.Relu,
                    bias=bi1_cols[:, i:i + 1], scale=float(wi1[h]),
                    accum_out=R_cat[:, N_FD + i:N_FD + i + 1])
            # junction casts (pad tile to 128 cols for the XBAR transpose)
            Rc16 = rcat_pool.tile([128, 128], F16, tag="Rc16")
            Rc16s.append(Rc16)
            nc.vector.tensor_copy(Rc16[:, 0:N_FD], R_cat[:, 0:N_FD])
            nc.scalar.copy(Rc16[:, N_FD:N_F], R_cat[:, N_FD:N_F])
            nc.vector.memset(Rc16[:, N_F:128], 0.0)

        def xbar_R(j):
            nc.sync.dma_start(out=R_fT[:, j * 128:(j + 1) * 128],
                              in_=Rc16s[j], transpose=True)

        # schedule: tile0 first so DVE/ACT start early
        emit_xp_mm(0)
        xp_cast(0)
        emit_fused(0)
        xbar_xp(0)
        xbar_R(0)
        for j in range(1, NT):
            emit_xp_mm(j)
            xp_cast(j)
            emit_fused(j)
            xbar_xp(j)
            xbar_R(j)

        # xpT pieces 2-3 via PE (runs in PE's idle window; bp added in cast)
        for p in (1, 2, 3):
            xpT_ps = ps_big.tile([128, BL], F32, tag="big", name=f"xpTps{p}")
            for c in range(KC):
                nc.tensor.matmul(
                    xpT_ps,
                    wp_sb[:, c * DOUT + p * 128:c * DOUT + (p + 1) * 128],
                    xT_sb[:, c * BL:(c + 1) * BL],
                    start=(c == 0), stop=(c == KC - 1))
            nc.vector.tensor_scalar(
                xpT_sb[:, p * BL:(p + 1) * BL], xpT_ps,
                bp4_cols[:, p:p + 1], 0.0, OP.add, OP.add)

        # transposed inner: DVE elementwise + PE reduce into uT_dup
        first_mm = [True]

        def uT_mm(lhsT, rhs, stop=False):
            nc.tensor.matmul(uT_ps, lhsT, rhs, start=first_mm[0], stop=stop,
                             skip_group_check=True)
            first_mm[0] = False

        for t, h in enumerate(H_TR):
            s_all = s_pool.tile([128, NT * BL], F16, tag="s_all")
            op1 = OP.max if wi1[h] > 0 else OP.min
            nc.vector.tensor_scalar(
                s_all, xpT_sb, float(c_in[h]), 0.0, OP.subtract, op1)
            for p in range(NT):
                uT_mm(w2d_sb[:, t * 128:(t + 1) * 128],
                      s_all[:, p * BL:(p + 1) * BL])

        # fused R route: one matmul over the XBAR'd transposes
        uT_mm(wi2fd_sb, R_fT[0:N_F, :], stop=True)

        # outer stage: u2 = f16(uT + bias_u) on ACT
        nc.scalar.activation(u2_sb, uT_ps, AF.Identity,
                             bias=biasud_col, scale=1.0)
        S_T_ps = ps_st.tile([H2, BL], F32, tag="S_T")
        pe_pairs = [jp for jp in range(npair) if jp >= N_GP_PAIRS]
        for jp, (ha, hb, use_max, b_valid) in enumerate(pairs):
            s2 = s2_pool.tile([128, BL], F16, tag="s2")
            op1 = OP.max if use_max else OP.min
            nc.vector.tensor_scalar(
                s2, u2_sb, cpair_cols[:, jp:jp + 1], 0.0, OP.subtract, op1)
            if jp < N_GP_PAIRS:
                # partition-reduce halves on GpSimd straight into S_T_sb
                nc.gpsimd.tensor_reduce(
                    S_T_sb[ha:ha + 1, :], s2[0:64, :],
                    mybir.AxisListType.C, OP.add)
                if b_valid:
                    nc.gpsimd.tensor_reduce(
                        S_T_sb[hb:hb + 1, :], s2[64:128, :],
                        mybir.AxisListType.C, OP.add)
            else:
                last = (jp == npair - 1)
                nc.tensor.matmul(S_T_ps,
                                 pmask_sb[:, jp * 64:(jp + 1) * 64], s2,
                                 start=(jp == pe_pairs[0]), stop=last)
        if N_GP_PAIRS == 0:
            nc.scalar.copy(S_T16[0:H2, :], S_T_ps)
        else:
            # move PE-reduced rows to S_T_sb (f32), then cast whole to f16
            gp_rows = set()
            for jp in range(min(N_GP_PAIRS, npair)):
                ha, hb, _, b_valid = pairs[jp]
                gp_rows.add(ha)
                if b_valid:
                    gp_rows.add(hb)
            pe_rows = [h for h in range(H2) if h not in gp_rows]
            runs = []
            for h in pe_rows:
                if runs and runs[-1][1] == h:
                    runs[-1][1] = h + 1
                else:
                    runs.append([h, h + 1])
            for a, b_ in runs:
                nc.scalar.copy(S_T_sb[a:b_, :], S_T_ps[a:b_, :])
            nc.vector.tensor_copy(S_T16[0:H2, :], S_T_sb[0:H2, :])

        # final matmuls + LayerNorm per row-tile
        for j in range(NT):
            sum_ps = ps_big.tile([128, DOUT], F32, tag="big")
            nc.tensor.matmul(sum_ps, S_T16[:, j * 128:(j + 1) * 128],
                             wo2e_sb, start=True, stop=True)
            st6 = small.tile([128, 6], F32, tag="st6")
            nc.vector.bn_stats(out=st6, in_=sum_ps)
            mv = small.tile([128, 2], F32, tag="mv")
            nc.vector.bn_aggr(out=mv, in_=st6)
            sig = small.tile([128, 1], F32, tag="sig")
            nc.scalar.activation(sig, mv[:, 1:2], AF.Sqrt,
                                 bias=eps_sb[:, 0:1], scale=1.0)
            r = small.tile([128, 1], F32, tag="r")
            nc.vector.reciprocal(r, sig)
            negmur = small.tile([128, 1], F32, tag="nmr")
            nc.vector.tensor_scalar(
                negmur, mv[:, 0:1], r[:, 0:1], -1.0, OP.mult, OP.mult)
            y_sb = ypool.tile([128, DOUT], F16, tag="y_sb")
            nc.scalar.activation(y_sb, sum_ps, AF.Identity,
                                 bias=negmur[:, 0:1], scale=r[:, 0:1])
            if not trivial_ln:
                yg = ypool.tile([128, DOUT], F32, tag="yg")
                nc.vector.scalar_tensor_tensor(
                    yg, y_sb, 1.0, gam_rep, OP.mult, OP.mult)
                y_sb2 = ypool.tile([128, DOUT], F32, tag="y_sb2")
                nc.vector.tensor_add(y_sb2, yg, bet_rep)
                y_sb = y_sb2
            nc.sync.dma_start(out=y[j * 128:(j + 1) * 128, :], in_=y_sb)

    return nc


def _split_waits(nc):
    count = 0
    for fn in nc.m.functions:
        for blk in fn.blocks:
            new_insts = []
            for inst in blk.instructions:
                si = getattr(inst, "sync_info", None)
                waits = list(si.on_wait) if si is not None and si.on_wait else []
                if len(waits) > 1:
                    for w in waits[:-1]:
                        count += 1
                        new_insts.append(mybir.InstNoOp(
                            name=f"I-waitnop-{count}",
                            engine=inst.engine,
                            ins=[], outs=[],
                            sync_info=mybir.SyncInfo(on_wait=[w], on_update=[]),
                        ))
                    si.on_wait = waits[-1:]
                new_insts.append(inst)
            blk.instructions = new_insts
    return count


def kernel(x, wp, bp, wi1, bi1, wi2, bi2, wo1, bo1, wo2, bo2, gamma, beta,
           _trace=False):
    f = lambda a: np.asarray(a, dtype=np.float32)
    x, wp = f(x), f(wp)
    bp, wi1, bi1, wi2, bi2 = f(bp), f(wi1), f(bi1), f(wi2), f(bi2)
    wo1, bo1, wo2, bo2, gamma, beta = (
        f(wo1), f(bo1), f(wo2), f(bo2), f(gamma), f(beta))

    trivial_ln = bool(np.allclose(gamma, 1.0) and np.allclose(beta, 0.0))
    pairs, c2 = _outer_pairs(wo1, bo1)
    npair = len(pairs)
    L = _c16_layout(npair)

    w_eff = wi2.copy()
    for h in H_FD + H_TR:
        w_eff[h] = wi1[h] * wi2[h]
    bias_u = DOUT * bi2.copy()
    for h in H_FD:
        bias_u += DOUT * bi1[h] * wi2[h]

    dup = lambda v: np.concatenate([v, v])
    c16buf = np.zeros((128, L["total"]), dtype=np.float16)
    for t, h in enumerate(H_TR):
        c16buf[:, L["w2dup"] + t * 128:L["w2dup"] + (t + 1) * 128] = \
            dup(w_eff[h])[None, :]
    for i, h in enumerate(H_FD + H_FA):
        c16buf[i, L["wi2fd"]:L["wi2fd"] + 128] = dup(w_eff[h])
    for jp, (ha, hb, use_max, b_valid) in enumerate(pairs):
        c16buf[0:64, L["pmask"] + jp * 64 + ha] = 1.0
        if b_valid:
            c16buf[64:128, L["pmask"] + jp * 64 + hb] = 1.0
    c16buf[0:H2, L["wo2e"]:L["wo2e"] + DOUT] = (wo1[:, None] * wo2)
    c16buf[H2, L["wo2e"]:L["wo2e"] + DOUT] = Q * bo2

    c32buf = np.zeros((128, 517 + npair), dtype=np.float32)
    c32buf[:, 0:DOUT] = bp[None, :]
    c32buf[:, 513 + npair:517 + npair] = bp.reshape(NT, 128).T
    c32buf[:, 512] = dup(bias_u)
    for jp, (ha, hb, use_max, b_valid) in enumerate(pairs):
        c32buf[0:64, 513 + jp] = c2[ha]
        c32buf[64:128, 513 + jp] = c2[hb]

    nc = _build_program(wi1, bi1, wo1, bo1, trivial_ln, pairs)
    _split_waits(nc)

    shared = {
        "wpall": np.ascontiguousarray(
            wp.reshape(KC, 128, DOUT).transpose(1, 0, 2).reshape(
                128, KC * DOUT)).astype(np.float16),
        "c16": c16buf, "c32": c32buf,
        "gamma": gamma, "beta": beta,
    }
    in_maps = []
    for i in range(NCORES):
        m = dict(shared)
        xTc = np.ascontiguousarray(x[i * BL:(i + 1) * BL, :].T)  # [768, 512]
        m["xTall"] = np.ascontiguousarray(
            xTc.reshape(KC, 128, BL).transpose(1, 0, 2).reshape(
                128, KC * BL)).astype(np.float16)
        in_maps.append(m)

    res = run_bass_kernel_spmd(nc, in_maps, core_ids=list(range(NCORES)),
                               trace=_trace)
    out = np.concatenate(
        [res.results[i]["y"].astype(np.float32) for i in range(NCORES)],
        axis=0)
    if _trace:
        kernel.last_result = res
    return out
